# revision 1
# baseline (speedup 1.0000x reference)
"""Causal (cumulative) LayerNorm Trainium2 Bass kernel.

Full-input contract: kernel(inputs, gamma, beta) takes the full
(B=8, K=16000, H=256) f32 tensor, shards batch across 8 NeuronCores
(one sample per core), and returns the full (8, 16000, 256) output.

Per-core algorithm (x is (K, H)):
  rowsum[k]   = sum_h x[k, h]
  rowsumsq[k] = sum_h x[k, h]^2
  csum = cumsum(rowsum); cpow = cumsum(rowsumsq)
  mean[k] = csum[k] / (H*(k+1));  msq[k] = cpow[k] / (H*(k+1))
  var[k] = msq[k] - mean[k]^2
  out[k, h] = gamma[h] * (x[k, h] - mean[k]) / sqrt(var[k] + EPS) + beta[h]

Layout: row k = b*3200 + p*25 + r for band b in 0..4, partition p in
0..127, r in 0..24. Each band is one (128, 25, 256) SBUF tile whose
per-partition 25 rows are CONTIGUOUS in HBM, so band loads/stores are
single DMA triggers with 25 KB contiguous runs per partition (max DMA
efficiency, negligible descriptor-gen on the sequencers).

Per band: one bn_stats per r (even/odd mean/M2 per row), six cheap
full-partition merges to rowsum/128 and rowsumsq, a fp32 vector scan
along r (prefix within each 25-row chunk), chunk totals prefix-summed
ACROSS partitions by one strictly-triangular PE matmul (PE fp32 is
full-precision, ~6e-8 verified) plus a broadcast matmul seeding the
running inter-band carry, then per-row rstd / -mean*rstd computed
directly in affine-ready layout (no transposes anywhere). Output pass
runs IN PLACE over the band tile: per-r affine (scalar engine mostly),
batched gamma multiply (gpsimd/vector), one store trigger per r-group.
Bands pipeline: band b's output overlaps band b+1's load. DMA is the
roofline: ~32.8 MB @ ~358 GB/s/core.
"""

import numpy as np

import concourse.bass as bass
import concourse.bacc as bacc
import concourse.tile as tile
from concourse import mybir
from concourse.bass_utils import run_bass_kernel_spmd

EPS = 1e-8
B, K, H = 8, 16000, 256
P = 128                  # SBUF partitions = chunks per band
CL = 25                  # rows per chunk (per partition per band)
BANDS = K // (P * CL)    # 5
G = 5                    # rows per gamma/store group
NGB = CL // G            # 5 groups per band
F32 = mybir.dt.float32
ALU = mybir.AluOpType
ACTF = mybir.ActivationFunctionType

GAMMA_DVE_J = {2}        # per-band group index -> gamma on vector, rest gpsimd
AFFINE_GPS_J = {1, 3}    # per-band group index -> affine on gpsimd, rest scalar


def _build(use_beta: bool):
    nc = bacc.Bacc("TRN2", target_bir_lowering=False, debug=False)

    x = nc.declare_dram_parameter("x", [K, H], F32, isOutput=False)
    gamma_b = nc.declare_dram_parameter("gamma_b", [P, H], F32, isOutput=False)
    beta_b = (
        nc.declare_dram_parameter("beta_b", [P, H], F32, isOutput=False)
        if use_beta
        else None
    )
    utri = nc.declare_dram_parameter("utri", [P, P], F32, isOutput=False)
    ident = nc.declare_dram_parameter("ident", [P, P], F32, isOutput=False)
    ones_col = nc.declare_dram_parameter("ones_col", [P, 1], F32, isOutput=False)
    ones_row = nc.declare_dram_parameter("ones_row", [1, P], F32, isOutput=False)
    invc_m = nc.declare_dram_parameter("invc_m", [P, BANDS, CL], F32, isOutput=False)
    invc_p = nc.declare_dram_parameter("invc_p", [P, BANDS, CL], F32, isOutput=False)
    y = nc.declare_dram_parameter("y", [K, H], F32, isOutput=True)

    xr = x.rearrange("(b p r) h -> b p r h", p=P, r=CL)   # [5, 128, 25, 256]
    yr = y.rearrange("(b p r) h -> b p r h", p=P, r=CL)

    with tile.TileContext(nc) as tc:
        with (
            tc.tile_pool(name="singles", bufs=1) as singles,
            tc.tile_pool(name="xband", bufs=BANDS) as xband,
            tc.tile_pool(name="opool", bufs=12) as opool,
            tc.tile_pool(name="segp", bufs=3) as segp,
            tc.tile_pool(name="psum", bufs=2, space="PSUM") as psum,
        ):
            sb_gamma = singles.tile([P, H], F32)
            nc.sync.dma_start(out=sb_gamma[:], in_=gamma_b[:])
            if use_beta:
                sb_beta = singles.tile([P, H], F32)
                nc.sync.dma_start(out=sb_beta[:], in_=beta_b[:])
            sb_utri = singles.tile([P, P], F32)
            nc.sync.dma_start(out=sb_utri[:], in_=utri[:])
            sb_ident = singles.tile([P, P], F32)
            nc.sync.dma_start(out=sb_ident[:], in_=ident[:])
            sb_onec = singles.tile([P, 1], F32)
            nc.sync.dma_start(out=sb_onec[:], in_=ones_col[:])
            sb_oner = singles.tile([1, P], F32)
            nc.sync.dma_start(out=sb_oner[:], in_=ones_row[:])
            sb_invm = singles.tile([P, BANDS, CL], F32)
            nc.sync.dma_start(out=sb_invm[:], in_=invc_m[:])
            sb_invp = singles.tile([P, BANDS, CL], F32)
            nc.sync.dma_start(out=sb_invp[:], in_=invc_p[:])

            sb_eps = singles.tile([P, 1], F32)
            nc.vector.memset(sb_eps[:], EPS)
            carry = singles.tile([1, 2], F32)
            nc.vector.memset(carry[:], 0.0)

            gamma_bc = sb_gamma[:].rearrange("p (o h) -> p o h", o=1).to_broadcast(
                (P, G, H)
            )
            if use_beta:
                beta_bc = sb_beta[:].rearrange("p (o h) -> p o h", o=1).to_broadcast(
                    (P, G, H)
                )

            xb = []
            invb = {}
            nmib = {}

            def load_band(b):
                xt = xband.tile([P, CL, H], F32)
                nsub = NGB if b == 0 else 1
                step = CL // nsub
                xv = xr[b]
                bnb = segp.tile([P, CL, 6], F32, tag="bn")
                for u in range(nsub):
                    nc.sync.dma_start(
                        out=xt[:, u * step:(u + 1) * step, :],
                        in_=xv[:, u * step:(u + 1) * step, :],
                    )
                    for r in range(u * step, (u + 1) * step):
                        nc.vector.bn_stats(out=bnb[:, r, :], in_=xt[:, r, :])
                xb.append(xt)
                return bnb

            def scan_band(b, bnb):
                me = bnb[:, :, 1]
                mo = bnb[:, :, 4]
                m2e = bnb[:, :, 2]
                m2o = bnb[:, :, 5]
                # rowsum/128: merge even/odd on the PE via identity-matmul
                # accumulation (I@me + I@mo); likewise rowsumsq picks up
                # m2e + m2o + 128*(me^2 + mo^2) in one PSUM accumulation.
                pe = segp.tile([P, CL], F32, tag="pe")
                nc.vector.scalar_tensor_tensor(
                    out=pe[:], in0=me, scalar=128.0, in1=me,
                    op0=ALU.mult, op1=ALU.mult,
                )
                po = segp.tile([P, CL], F32, tag="po")
                nc.vector.scalar_tensor_tensor(
                    out=po[:], in0=mo, scalar=128.0, in1=mo,
                    op0=ALU.mult, op1=ALU.mult,
                )
                se_ps = psum.tile([P, CL], F32, tag="se_ps")
                nc.tensor.matmul(
                    se_ps[:], lhsT=sb_ident[:], rhs=me, start=True, stop=False
                )
                nc.tensor.matmul(
                    se_ps[:], lhsT=sb_ident[:], rhs=mo, start=False, stop=True
                )
                sp_ps = psum.tile([P, CL], F32, tag="sp_ps")
                nc.tensor.matmul(
                    sp_ps[:], lhsT=sb_ident[:], rhs=m2e, start=True, stop=False
                )
                nc.tensor.matmul(
                    sp_ps[:], lhsT=sb_ident[:], rhs=m2o, start=False, stop=False
                )
                nc.tensor.matmul(
                    sp_ps[:], lhsT=sb_ident[:], rhs=pe[:], start=False, stop=False
                )
                nc.tensor.matmul(
                    sp_ps[:], lhsT=sb_ident[:], rhs=po[:], start=False, stop=True
                )

                # prefix along r within each chunk
                scan_s = segp.tile([P, CL], F32, tag="scan_s")
                nc.vector.tensor_tensor_scan(
                    out=scan_s[:], data0=se_ps[:], data1=pe[:],
                    initial=0.0, op0=ALU.add, op1=ALU.bypass,
                )
                scan_p = segp.tile([P, CL], F32, tag="scan_p")
                nc.vector.tensor_tensor_scan(
                    out=scan_p[:], data0=sp_ps[:], data1=pe[:],
                    initial=0.0, op0=ALU.add, op1=ALU.bypass,
                )

                # chunk totals -> exclusive prefix across partitions (PE)
                tot = segp.tile([P, 2], F32, tag="tot")
                nc.vector.tensor_copy(out=tot[:, 0:1], in_=scan_s[:, CL - 1:CL])
                nc.vector.tensor_copy(out=tot[:, 1:2], in_=scan_p[:, CL - 1:CL])
                offs = psum.tile([P, 2], F32, tag="offs")
                nc.tensor.matmul(
                    offs[:], lhsT=sb_utri[:], rhs=tot[:], start=True, stop=False
                )
                nc.tensor.matmul(
                    offs[:], lhsT=sb_oner[:], rhs=carry[:], start=False, stop=True
                )
                # band total (1,2) for the running carry
                btot = psum.tile([1, 2], F32, tag="btot")
                nc.tensor.matmul(
                    btot[:], lhsT=sb_onec[:], rhs=tot[:], start=True, stop=True
                )
                nc.vector.tensor_add(out=carry[:], in0=carry[:], in1=btot[:])

                # mean / msq / var / rstd / -mean*rstd  (affine-ready layout)
                mean_c = segp.tile([P, CL], F32, tag="mean_c")
                nc.vector.scalar_tensor_tensor(
                    out=mean_c[:], in0=scan_s[:], scalar=offs[:, 0:1],
                    in1=sb_invm[:, b, :], op0=ALU.add, op1=ALU.mult,
                )
                msq_c = segp.tile([P, CL], F32, tag="msq_c")
                nc.vector.scalar_tensor_tensor(
                    out=msq_c[:], in0=scan_p[:], scalar=offs[:, 1:2],
                    in1=sb_invp[:, b, :], op0=ALU.add, op1=ALU.mult,
                )
                var_c = segp.tile([P, CL], F32, tag="var_c")
                nc.vector.tensor_mul(out=var_c[:], in0=mean_c[:], in1=mean_c[:])
                nc.vector.tensor_sub(out=var_c[:], in0=msq_c[:], in1=var_c[:])
                sd_c = segp.tile([P, CL], F32, tag="sd_c")
                nc.scalar.activation(
                    out=sd_c[:], in_=var_c[:], func=ACTF.Sqrt, bias=sb_eps[:],
                )
                inv_c = segp.tile([P, CL], F32, tag="inv_c")
                nc.vector.reciprocal(out=inv_c[:], in_=sd_c[:])
                nmi_c = segp.tile([P, CL], F32, tag="nmi_c")
                nc.vector.scalar_tensor_tensor(
                    out=nmi_c[:], in0=mean_c[:], scalar=-1.0, in1=inv_c[:],
                    op0=ALU.mult, op1=ALU.mult,
                )
                invb[b] = inv_c
                nmib[b] = nmi_c

            obands = {}

            # last band runs after all loads: vector is idle then, so its
            # output pass spreads across all three engines
            LAST_AFF = ["act", "gps", "dve", "act", "dve"]
            LAST_GAM = ["dve", "gps", "dve", "gps", "dve"]

            def out_band(b):
                # affine + gamma interleaved per 5-row group so gammas
                # pipeline group-by-group behind the affines
                xt = xb[b]
                inv_c = invb[b]
                nmi_c = nmib[b]
                last = b == BANDS - 1
                obs = []
                for j in range(NGB):
                    ob = opool.tile([P, G, H], F32)
                    if last:
                        aeng = LAST_AFF[j]
                    else:
                        aeng = "gps" if j in AFFINE_GPS_J else "act"
                    for jr in range(G):
                        r = j * G + jr
                        if aeng == "act":
                            nc.scalar.activation(
                                out=ob[:, jr, :], in_=xt[:, r, :],
                                func=ACTF.Identity,
                                bias=nmi_c[:, r:r + 1], scale=inv_c[:, r:r + 1],
                            )
                        else:
                            eng = nc.gpsimd if aeng == "gps" else nc.vector
                            eng.tensor_scalar(
                                out=ob[:, jr, :], in0=xt[:, r, :],
                                scalar1=inv_c[:, r:r + 1],
                                scalar2=nmi_c[:, r:r + 1],
                                op0=ALU.mult, op1=ALU.add,
                            )
                    if last:
                        geng = nc.vector if LAST_GAM[j] == "dve" else nc.gpsimd
                    else:
                        geng = nc.vector if j in GAMMA_DVE_J else nc.gpsimd
                    geng.tensor_mul(out=ob[:], in0=ob[:], in1=gamma_bc)
                    if use_beta:
                        geng.tensor_add(out=ob[:], in0=ob[:], in1=beta_bc)
                    obs.append(ob)
                obands[b] = obs

            def store_band(b):
                for j, ob in enumerate(obands[b]):
                    nc.scalar.dma_start(
                        out=yr[b][:, j * G:(j + 1) * G, :], in_=ob[:],
                    )

            # pipelined emission, output lagging one band. Affines come
            # before the next chain in the ACT stream; vector gammas come
            # after the chain in the DVE stream (so the chain never waits
            # behind a gamma that itself waits on ACT affines); store
            # triggers last.
            bn0 = load_band(0)
            scan_band(0, bn0)
            for b in range(1, BANDS):
                bnb = load_band(b)
                out_band(b - 1)
                scan_band(b, bnb)
                store_band(b - 1)
            out_band(BANDS - 1)
            store_band(BANDS - 1)

    nc.compile()
    return nc


_CACHE = {}


def _get(use_beta: bool):
    if use_beta not in _CACHE:
        _CACHE[use_beta] = _build(use_beta)
    return _CACHE[use_beta]


def _make_consts():
    # strictly-upper triangular ones: lhsT[q, p] = 1 iff q < p
    utri = np.triu(np.ones((P, P), dtype=np.float32), k=1)
    ident = np.eye(P, dtype=np.float32)
    ones_col = np.ones((P, 1), dtype=np.float32)
    ones_row = np.ones((1, P), dtype=np.float32)
    k = np.arange(K, dtype=np.float64).reshape(BANDS, P, CL)  # [b, p, r]
    counts = np.transpose(k, (1, 0, 2)) + 1.0                 # [p, b, r]
    invc_m = (1.0 / (2.0 * counts)).astype(np.float32)
    invc_p = (1.0 / (float(H) * counts)).astype(np.float32)
    return utri, ident, ones_col, ones_row, invc_m, invc_p


def _prepare(inputs, gamma, beta):
    inputs = np.ascontiguousarray(inputs, dtype=np.float32)
    gamma = np.asarray(gamma, dtype=np.float32).reshape(1, H)
    beta = np.asarray(beta, dtype=np.float32).reshape(1, H)
    use_beta = bool(np.any(beta))

    gamma_b = np.ascontiguousarray(np.broadcast_to(gamma, (P, H)))
    utri, ident, ones_col, ones_row, invc_m, invc_p = _make_consts()

    in_maps = []
    for b in range(B):
        m = {
            "x": np.ascontiguousarray(inputs[b]),
            "gamma_b": gamma_b,
            "utri": utri,
            "ident": ident,
            "ones_col": ones_col,
            "ones_row": ones_row,
            "invc_m": invc_m,
            "invc_p": invc_p,
        }
        if use_beta:
            m["beta_b"] = np.ascontiguousarray(np.broadcast_to(beta, (P, H)))
        in_maps.append(m)
    return use_beta, in_maps


def kernel(inputs: np.ndarray, gamma: np.ndarray, beta: np.ndarray) -> np.ndarray:
    use_beta, in_maps = _prepare(inputs, gamma, beta)
    nc = _get(use_beta)
    res = run_bass_kernel_spmd(nc, in_maps, list(range(B)))
    out = np.stack([res.results[b]["y"] for b in range(B)], axis=0)
    return out



# revision 2
# speedup vs baseline: 1.0555x; 1.0555x over previous
"""Causal (cumulative) LayerNorm Trainium2 Bass kernel — bf16 edition.

Full-input contract: kernel(inputs, gamma, beta) takes the full
(B=8, K=16000, H=256) f32 tensor, shards batch across 8 NeuronCores
(one sample per core), and returns the full (8, 16000, 256) f32 output.
Inputs are converted to bf16 on the host (rel tolerance 2e-2 >> bf16
rounding); outputs come back bf16 and are upcast on the host. This
halves HBM traffic (16.4 MB/core) and doubles DVE throughput on the
elementwise passes.

Per-core algorithm (x is (K, H)):
  rowmean[k] = mean_h x[k, h];  rowM2[k] = sum_h (x[k,h]-rowmean[k])^2
  csum = cumsum(rowmean); cq = cumsum(rowM2 + H*rowmean^2)
  mean[k] = csum[k]/(k+1);  msq[k] = cq[k]/(H*(k+1))
  var = msq - mean^2; out = gamma * (x - mean) * rsqrt(var+EPS) + beta

Layout: row k = b*3200 + p*25 + r, band b in 0..4 = one (128, 25, 256)
SBUF tile, 12.8 KB contiguous per partition per band DMA.

Stats: 13 bn_stats per band, each consuming TWO rows through an
h-outer/row-inner interleaved AP so the engine's even/odd split yields
independent per-row (mean, M2) — hardware-verified. The bn output
buffer is [128, 26, 3] = (count, mean, M2) per row, so all 26 row
means (25 real + 1 dup) form a single stride-3 [128, 26] AP: the
within-chunk prefix scans read it directly. Chunk totals are
prefix-summed across partitions by a strictly-triangular PE matmul
plus a ones-row matmul seeding the inter-band carry (PE fp32).

Output pass (in place over the band tile): per-row affine on ScalarE
(rows 0-18) and GpSimd (19-24), gamma broadcast-multiply merged-TT on
DVE (rows 0-8, 2x bf16 mode) and GpSimd (8-25). The last band, which
has no following stats work, spreads the affine over DVE via a
paired-scalar broadcast TT (2x mode) to shorten the drain.
Engine budget/band ~12 us each on DVE/ACT/GPS; DMA ~9.2 us.
"""

import numpy as np
import ml_dtypes

import concourse.bass as bass
import concourse.bacc as bacc
import concourse.tile as tile
from concourse import mybir
from concourse.bass_utils import run_bass_kernel_spmd

EPS = 1e-8
B, K, H = 8, 16000, 256
P = 128                  # SBUF partitions = chunks per band
CL = 25                  # rows per chunk (per partition per band)
BANDS = K // (P * CL)    # 5
NPAIR = 13               # bn_stats ops per band (12 pairs + 1 dup-pair)
F32 = mybir.dt.float32
BF16 = mybir.dt.bfloat16
ALU = mybir.AluOpType
ACTF = mybir.ActivationFunctionType

# steady-state row splits (tuned to balance DVE/ACT/GPS at ~12 us/band)
ACT_AFF_END = 19         # affine rows [0, 19) on ScalarE, [19, 25) on GpSimd
DVE_GAM_END = 8          # gamma rows [0, 8) on DVE, [8, 25) on GpSimd
GPS_GAM_SPLITS = [(8, 14), (14, 20), (20, 25)]
# last band: no following stats, so DVE takes a big share
L_ACT_AFF_END = 13       # ACT affine rows [0, 13), DVE merged affine [13, 25)
L_DVE_GAM_END = 13
L_GPS_GAM_SPLITS = [(13, 19), (19, 25)]
STORE_SPLIT = 8          # store chunks [0, split) and [split, 25)


def _bn_stats_raw(nc, out, in_):
    """bn_stats with an arbitrary input AP (bypasses the bass wrapper's
    output-shape rule). HW streams the AP; even/odd stream positions get
    independent (count, mean, M2). out must total 6 elems/partition."""
    v = nc.vector
    return v.add_instruction(
        mybir.InstBNStats(
            name=v.bass.get_next_instruction_name(),
            ins=[v.lower_ap(in_)],
            outs=[v.lower_ap(out)],
        )
    )


def _build(use_beta: bool):
    nc = bacc.Bacc("TRN2", target_bir_lowering=False, debug=False)

    x = nc.declare_dram_parameter("x", [K, H], BF16, isOutput=False)
    gamma_b = nc.declare_dram_parameter("gamma_b", [P, H], BF16, isOutput=False)
    beta_b = (
        nc.declare_dram_parameter("beta_b", [P, H], BF16, isOutput=False)
        if use_beta
        else None
    )
    utri = nc.declare_dram_parameter("utri", [P, P], F32, isOutput=False)
    ones_col = nc.declare_dram_parameter("ones_col", [P, 1], F32, isOutput=False)
    ones_row = nc.declare_dram_parameter("ones_row", [1, P], F32, isOutput=False)
    invc_m = nc.declare_dram_parameter("invc_m", [P, BANDS, CL], F32, isOutput=False)
    invc_p = nc.declare_dram_parameter("invc_p", [P, BANDS, CL], F32, isOutput=False)
    y = nc.declare_dram_parameter("y", [K, H], BF16, isOutput=True)

    xr = x.rearrange("(b p r) h -> b p r h", p=P, r=CL)   # [5, 128, 25, 256]
    yr = y.rearrange("(b p r) h -> b p r h", p=P, r=CL)

    with tile.TileContext(nc) as tc:
        with (
            tc.tile_pool(name="singles", bufs=1) as singles,
            tc.tile_pool(name="xband", bufs=BANDS) as xband,
            tc.tile_pool(name="sb", bufs=3) as sb,
            tc.tile_pool(name="psum", bufs=4, space="PSUM") as psum,
        ):
            sb_gamma = singles.tile([P, H], BF16)
            nc.sync.dma_start(out=sb_gamma[:], in_=gamma_b[:])
            if use_beta:
                sb_beta = singles.tile([P, H], BF16)
                nc.sync.dma_start(out=sb_beta[:], in_=beta_b[:])
            sb_utri = singles.tile([P, P], F32)
            nc.sync.dma_start(out=sb_utri[:], in_=utri[:])
            sb_onec = singles.tile([P, 1], F32)
            nc.sync.dma_start(out=sb_onec[:], in_=ones_col[:])
            sb_oner = singles.tile([1, P], F32)
            nc.sync.dma_start(out=sb_oner[:], in_=ones_row[:])
            sb_invm = singles.tile([P, BANDS, CL], F32)
            nc.sync.dma_start(out=sb_invm[:], in_=invc_m[:])
            sb_invp = singles.tile([P, BANDS, CL], F32)
            nc.sync.dma_start(out=sb_invp[:], in_=invc_p[:])

            sb_eps = singles.tile([P, 1], F32)
            nc.vector.memset(sb_eps[:], EPS)
            carry = singles.tile([1, 2], F32)
            nc.vector.memset(carry[:], 0.0)

            def gamma_bc(n):
                return sb_gamma[:].rearrange("p (o h) -> p o h", o=1).to_broadcast(
                    (P, n, H)
                )

            def beta_bc(n):
                return sb_beta[:].rearrange("p (o h) -> p o h", o=1).to_broadcast(
                    (P, n, H)
                )

            xb = {}
            invb = {}
            nmib = {}
            varb = {}
            meanb = {}

            def load_band(b):
                xt = xband.tile([P, CL, H], BF16)
                nc.sync.dma_start(out=xt[:], in_=xr[b])
                xb[b] = xt

            def stats_dve(b):
                """bn pairs + scans + PE cumsum + mean/msq/var (no sqrt)."""
                xt = xb[b]
                bnb = sb.tile([P, 2 * NPAIR, 3], F32, tag="bnb")
                for i in range(12):
                    _bn_stats_raw(
                        nc,
                        bnb[:, 2 * i:2 * i + 2, :],
                        xt[:, 2 * i:2 * i + 2, :].rearrange("p r h -> p h r"),
                    )
                # row 24 duplicated into both stream phases (stride-0 pair)
                _bn_stats_raw(
                    nc,
                    bnb[:, 24:26, :],
                    xt[:, 24:25, :]
                    .rearrange("p r h -> p h r")
                    .to_broadcast((P, H, 2)),
                )
                meansv = bnb[:, :, 1]        # [128, 26] stride-3: per-row means
                m2v = bnb[:, :, 2]           # [128, 26]: per-row M2
                ut = sb.tile([P, 2 * NPAIR], F32, tag="ut")
                nc.vector.tensor_tensor(out=ut[:], in0=meansv, in1=meansv,
                                        op=ALU.mult)
                qt = sb.tile([P, 2 * NPAIR], F32, tag="qt")
                nc.vector.scalar_tensor_tensor(
                    out=qt[:], in0=ut[:], scalar=float(H), in1=m2v,
                    op0=ALU.mult, op1=ALU.add,
                )
                St = sb.tile([P, 2 * NPAIR], F32, tag="St")
                nc.vector.tensor_tensor_scan(
                    out=St[:], data0=meansv, data1=meansv,
                    initial=0.0, op0=ALU.add, op1=ALU.bypass,
                )
                Qt = sb.tile([P, 2 * NPAIR], F32, tag="Qt")
                nc.vector.tensor_tensor_scan(
                    out=Qt[:], data0=qt[:], data1=qt[:],
                    initial=0.0, op0=ALU.add, op1=ALU.bypass,
                )
                tot = sb.tile([P, 2], F32, tag="tot")
                nc.vector.tensor_copy(out=tot[:, 0:1], in_=St[:, CL - 1:CL])
                nc.vector.tensor_copy(out=tot[:, 1:2], in_=Qt[:, CL - 1:CL])
                offs = psum.tile([P, 2], F32, tag="offs")
                nc.tensor.matmul(
                    offs[:], lhsT=sb_utri[:], rhs=tot[:], start=True, stop=False
                )
                nc.tensor.matmul(
                    offs[:], lhsT=sb_oner[:], rhs=carry[:], start=False, stop=True
                )
                btot = psum.tile([1, 2], F32, tag="btot")
                nc.tensor.matmul(
                    btot[:], lhsT=sb_onec[:], rhs=tot[:], start=True, stop=True
                )
                nc.vector.tensor_add(out=carry[:], in0=carry[:], in1=btot[:])

                mean = sb.tile([P, CL], F32, tag="mean")
                nc.vector.tensor_tensor(
                    out=mean[:], in0=St[:, 0:CL],
                    in1=offs[:, 0:1].to_broadcast((P, CL)), op=ALU.add,
                )
                nc.vector.tensor_tensor(
                    out=mean[:], in0=mean[:], in1=sb_invm[:, b, :], op=ALU.mult
                )
                msq = sb.tile([P, CL], F32, tag="msq")
                nc.vector.tensor_tensor(
                    out=msq[:], in0=Qt[:, 0:CL],
                    in1=offs[:, 1:2].to_broadcast((P, CL)), op=ALU.add,
                )
                nc.vector.tensor_tensor(
                    out=msq[:], in0=msq[:], in1=sb_invp[:, b, :], op=ALU.mult
                )
                var = sb.tile([P, CL], F32, tag="var")
                nc.vector.tensor_tensor(out=var[:], in0=mean[:], in1=mean[:],
                                        op=ALU.mult)
                nc.vector.tensor_tensor(out=var[:], in0=msq[:], in1=var[:],
                                        op=ALU.subtract)
                varb[b] = var
                meanb[b] = mean

            def stats_act(b):
                """sqrt (ScalarE) + reciprocal + nmi (DVE)."""
                sd = sb.tile([P, CL], F32, tag="sd")
                nc.scalar.activation(
                    out=sd[:], in_=varb[b][:], func=ACTF.Sqrt, bias=sb_eps[:],
                )
                inv = sb.tile([P, CL], F32, tag="inv")
                nc.vector.reciprocal(out=inv[:], in_=sd[:])
                nmi = sb.tile([P, CL], F32, tag="nmi")
                nc.vector.scalar_tensor_tensor(
                    out=nmi[:], in0=meanb[b][:], scalar=-1.0, in1=inv[:],
                    op0=ALU.mult, op1=ALU.mult,
                )
                invb[b] = inv
                nmib[b] = nmi

            def out_band(b):
                xt = xb[b]
                inv = invb[b]
                nmi = nmib[b]
                last = b == BANDS - 1
                act_end = L_ACT_AFF_END if last else ACT_AFF_END
                dve_gam_end = L_DVE_GAM_END if last else DVE_GAM_END
                gps_gams = L_GPS_GAM_SPLITS if last else GPS_GAM_SPLITS

                for r in range(act_end):
                    nc.scalar.activation(
                        out=xt[:, r, :], in_=xt[:, r, :], func=ACTF.Identity,
                        bias=nmi[:, r:r + 1], scale=inv[:, r:r + 1],
                    )
                if last:
                    # DVE merged affine rows [13, 25) via paired-scalar bcast
                    n = CL - act_end
                    inv2 = sb.tile([P, n, 2], BF16, tag="inv2")
                    nc.vector.tensor_copy(
                        out=inv2[:],
                        in_=inv[:, act_end:CL]
                        .rearrange("p (r o) -> p r o", o=1)
                        .to_broadcast((P, n, 2)),
                    )
                    nmi2 = sb.tile([P, n, 2], BF16, tag="nmi2")
                    nc.vector.tensor_copy(
                        out=nmi2[:],
                        in_=nmi[:, act_end:CL]
                        .rearrange("p (r o) -> p r o", o=1)
                        .to_broadcast((P, n, 2)),
                    )
                    xv = xt[:, act_end:CL, :].rearrange(
                        "p r (q j) -> p r q j", j=2
                    )
                    iv = inv2[:].rearrange("p r (o j) -> p r o j", o=1)\
                        .to_broadcast((P, n, H // 2, 2))
                    nv = nmi2[:].rearrange("p r (o j) -> p r o j", o=1)\
                        .to_broadcast((P, n, H // 2, 2))
                    nc.vector.tensor_tensor(out=xv, in0=xv, in1=iv, op=ALU.mult)
                    nc.vector.tensor_tensor(out=xv, in0=xv, in1=nv, op=ALU.add)
                else:
                    for r in range(act_end, CL):
                        nc.gpsimd.tensor_scalar(
                            out=xt[:, r, :], in0=xt[:, r, :],
                            scalar1=inv[:, r:r + 1], scalar2=nmi[:, r:r + 1],
                            op0=ALU.mult, op1=ALU.add,
                        )

                # gamma
                n = dve_gam_end
                nc.vector.tensor_tensor(
                    out=xt[:, 0:n, :], in0=xt[:, 0:n, :], in1=gamma_bc(n),
                    op=ALU.mult,
                )
                if use_beta:
                    nc.vector.tensor_tensor(
                        out=xt[:, 0:n, :], in0=xt[:, 0:n, :], in1=beta_bc(n),
                        op=ALU.add,
                    )
                for lo, hi in gps_gams:
                    nc.gpsimd.tensor_tensor(
                        out=xt[:, lo:hi, :], in0=xt[:, lo:hi, :],
                        in1=gamma_bc(hi - lo), op=ALU.mult,
                    )
                    if use_beta:
                        nc.gpsimd.tensor_tensor(
                            out=xt[:, lo:hi, :], in0=xt[:, lo:hi, :],
                            in1=beta_bc(hi - lo), op=ALU.add,
                        )

            def store_band(b):
                xt = xb[b]
                s = STORE_SPLIT
                nc.scalar.dma_start(out=yr[b][:, 0:s, :], in_=xt[:, 0:s, :])
                nc.scalar.dma_start(out=yr[b][:, s:CL, :], in_=xt[:, s:CL, :])

            for b in range(BANDS):
                load_band(b)
            stats_dve(0)
            stats_act(0)
            for b in range(1, BANDS):
                stats_dve(b)
                out_band(b - 1)
                stats_act(b)
                store_band(b - 1)
            out_band(BANDS - 1)
            store_band(BANDS - 1)

    nc.compile()
    return nc


_CACHE = {}


def _get(use_beta: bool):
    if use_beta not in _CACHE:
        _CACHE[use_beta] = _build(use_beta)
    return _CACHE[use_beta]


def _make_consts():
    # strictly-upper triangular ones: lhsT[q, p] = 1 iff q < p
    utri = np.triu(np.ones((P, P), dtype=np.float32), k=1)
    ones_col = np.ones((P, 1), dtype=np.float32)
    ones_row = np.ones((1, P), dtype=np.float32)
    k = np.arange(K, dtype=np.float64).reshape(BANDS, P, CL)  # [b, p, r]
    counts = np.transpose(k, (1, 0, 2)) + 1.0                 # [p, b, r]
    invc_m = (1.0 / counts).astype(np.float32)
    invc_p = (1.0 / (float(H) * counts)).astype(np.float32)
    return utri, ones_col, ones_row, invc_m, invc_p


def _prepare(inputs, gamma, beta):
    inputs = np.asarray(inputs, dtype=np.float32)
    gamma = np.asarray(gamma, dtype=np.float32).reshape(1, H)
    beta = np.asarray(beta, dtype=np.float32).reshape(1, H)
    use_beta = bool(np.any(beta))

    xbf = inputs.astype(ml_dtypes.bfloat16)
    gamma_b = np.ascontiguousarray(
        np.broadcast_to(gamma, (P, H))
    ).astype(ml_dtypes.bfloat16)
    utri, ones_col, ones_row, invc_m, invc_p = _make_consts()

    in_maps = []
    for b in range(B):
        m = {
            "x": np.ascontiguousarray(xbf[b]),
            "gamma_b": gamma_b,
            "utri": utri,
            "ones_col": ones_col,
            "ones_row": ones_row,
            "invc_m": invc_m,
            "invc_p": invc_p,
        }
        if use_beta:
            m["beta_b"] = np.ascontiguousarray(
                np.broadcast_to(beta, (P, H))
            ).astype(ml_dtypes.bfloat16)
        in_maps.append(m)
    return use_beta, in_maps


def kernel(inputs: np.ndarray, gamma: np.ndarray, beta: np.ndarray) -> np.ndarray:
    use_beta, in_maps = _prepare(inputs, gamma, beta)
    nc = _get(use_beta)
    res = run_bass_kernel_spmd(nc, in_maps, list(range(B)))
    out = np.stack(
        [np.asarray(res.results[b]["y"]).astype(np.float32) for b in range(B)],
        axis=0,
    )
    return out


# revision 5
# speedup vs baseline: 1.1449x; 1.0847x over previous
"""Causal (cumulative) LayerNorm Trainium2 Bass kernel — bf16 edition.

Full-input contract: kernel(inputs, gamma, beta) takes the full
(B=8, K=16000, H=256) f32 tensor, shards batch across 8 NeuronCores
(one sample per core), and returns the full (8, 16000, 256) f32 output.
Inputs are converted to bf16 on the host (rel tolerance 2e-2 >> bf16
rounding); outputs come back bf16 and are upcast on the host. This
halves HBM traffic (16.4 MB/core) and doubles DVE throughput on the
elementwise passes.

Per-core algorithm (x is (K, H)):
  rowmean[k] = mean_h x[k, h];  rowM2[k] = sum_h (x[k,h]-rowmean[k])^2
  csum = cumsum(rowmean); cq = cumsum(rowM2 + H*rowmean^2)
  mean[k] = csum[k]/(k+1);  msq[k] = cq[k]/(H*(k+1))
  var = msq - mean^2; out = gamma * (x - mean) * rsqrt(var+EPS) + beta

Layout: row k = b*3200 + p*25 + r, band b in 0..4 = one (128, 25, 256)
SBUF tile, 12.8 KB contiguous per partition per band DMA.

Stats: 13 bn_stats per band, each consuming TWO rows through an
h-outer/row-inner interleaved AP so the engine's even/odd split yields
independent per-row (mean, M2) — hardware-verified. The bn output
buffer is [128, 26, 3] = (count, mean, M2) per row, so all 26 row
means (25 real + 1 dup) form a single stride-3 [128, 26] AP: the
within-chunk prefix scans read it directly. Chunk totals are
prefix-summed across partitions by a strictly-triangular PE matmul
plus a ones-row matmul seeding the inter-band carry (PE fp32).

Output pass (in place over the band tile): per-row affine on ScalarE
(rows 0-18) and GpSimd (19-24), gamma broadcast-multiply merged-TT on
DVE (rows 0-8, 2x bf16 mode) and GpSimd (8-25). The last band, which
has no following stats work, spreads the affine over DVE via a
paired-scalar broadcast TT (2x mode) to shorten the drain.
Engine budget/band ~12 us each on DVE/ACT/GPS; DMA ~9.2 us.
"""

import numpy as np
import ml_dtypes

import concourse.bass as bass
import concourse.bacc as bacc
import concourse.tile as tile
from concourse import mybir
from concourse.bass_utils import run_bass_kernel_spmd

EPS = 1e-8
B, K, H = 8, 16000, 256
P = 128                  # SBUF partitions = chunks per band
CL = 25                  # rows per chunk (per partition per band)
BANDS = K // (P * CL)    # 5
NPAIR = 13               # bn_stats ops per band (12 pairs + 1 dup-pair)
F32 = mybir.dt.float32
BF16 = mybir.dt.bfloat16
ALU = mybir.AluOpType
ACTF = mybir.ActivationFunctionType

# steady-state row splits (tuned to balance DVE/ACT/GPS at ~12 us/band)
ACT_AFF_END = 19         # affine rows [0, 19) on ScalarE, [19, 25) on GpSimd
DVE_GAM_END = 8          # gamma rows [0, 8) on DVE, [8, 25) on GpSimd
GPS_GAM_SPLITS = [(8, 14), (14, 20), (20, 25)]
# last band: no following stats, so DVE takes a big share
L_ACT_AFF_END = 13       # ACT affine rows [0, 13), DVE merged affine [13, 25)
L_DVE_GAM_END = 13
L_GPS_GAM_SPLITS = [(13, 19), (19, 25)]
STORE_SPLIT = 8          # store chunks [0, split) and [split, 25)


def _bn_stats_raw(nc, out, in_):
    """bn_stats with an arbitrary input AP (bypasses the bass wrapper's
    output-shape rule). HW streams the AP; even/odd stream positions get
    independent (count, mean, M2). out must total 6 elems/partition."""
    v = nc.vector
    return v.add_instruction(
        mybir.InstBNStats(
            name=v.bass.get_next_instruction_name(),
            ins=[v.lower_ap(in_)],
            outs=[v.lower_ap(out)],
        )
    )


def _build(use_beta: bool):
    nc = bacc.Bacc("TRN2", target_bir_lowering=False, debug=False)

    x = nc.declare_dram_parameter("x", [K, H], BF16, isOutput=False)
    gamma_b = nc.declare_dram_parameter("gamma_b", [P, H], BF16, isOutput=False)
    beta_b = (
        nc.declare_dram_parameter("beta_b", [P, H], BF16, isOutput=False)
        if use_beta
        else None
    )
    utri = nc.declare_dram_parameter("utri", [P, P], F32, isOutput=False)
    ones_col = nc.declare_dram_parameter("ones_col", [P, 1], F32, isOutput=False)
    ones_row = nc.declare_dram_parameter("ones_row", [1, P], F32, isOutput=False)
    invc_m = nc.declare_dram_parameter("invc_m", [P, BANDS, CL], F32, isOutput=False)
    invc_p = nc.declare_dram_parameter("invc_p", [P, BANDS, CL], F32, isOutput=False)
    y = nc.declare_dram_parameter("y", [K, H], BF16, isOutput=True)

    xr = x.rearrange("(b p r) h -> b p r h", p=P, r=CL)   # [5, 128, 25, 256]
    yr = y.rearrange("(b p r) h -> b p r h", p=P, r=CL)

    with tile.TileContext(nc) as tc:
        with (
            tc.tile_pool(name="singles", bufs=1) as singles,
            tc.tile_pool(name="xband", bufs=BANDS) as xband,
            tc.tile_pool(name="sb", bufs=3) as sb,
            tc.tile_pool(name="psum", bufs=4, space="PSUM") as psum,
        ):
            sb_gamma = singles.tile([P, H], BF16)
            nc.sync.dma_start(out=sb_gamma[:], in_=gamma_b[:])
            if use_beta:
                sb_beta = singles.tile([P, H], BF16)
                nc.sync.dma_start(out=sb_beta[:], in_=beta_b[:])
            sb_utri = singles.tile([P, P], F32)
            nc.sync.dma_start(out=sb_utri[:], in_=utri[:])
            sb_onec = singles.tile([P, 1], F32)
            nc.sync.dma_start(out=sb_onec[:], in_=ones_col[:])
            sb_oner = singles.tile([1, P], F32)
            nc.sync.dma_start(out=sb_oner[:], in_=ones_row[:])
            sb_invm = singles.tile([P, BANDS, CL], F32)
            nc.sync.dma_start(out=sb_invm[:], in_=invc_m[:])
            sb_invp = singles.tile([P, BANDS, CL], F32)
            nc.sync.dma_start(out=sb_invp[:], in_=invc_p[:])

            sb_eps = singles.tile([P, 1], F32)
            nc.vector.memset(sb_eps[:], EPS)
            carry = singles.tile([1, 2], F32)
            nc.vector.memset(carry[:], 0.0)

            def gamma_bc(n):
                return sb_gamma[:].rearrange("p (o h) -> p o h", o=1).to_broadcast(
                    (P, n, H)
                )

            def beta_bc(n):
                return sb_beta[:].rearrange("p (o h) -> p o h", o=1).to_broadcast(
                    (P, n, H)
                )

            xb = {}
            invb = {}
            nmib = {}
            varb = {}
            meanb = {}

            def load_band(b, chunks=1):
                xt = xband.tile([P, CL, H], BF16)
                if chunks == 1:
                    nc.sync.dma_start(out=xt[:], in_=xr[b])
                else:
                    bounds = [0, 6, 12, 18, CL]
                    for lo, hi in zip(bounds[:-1], bounds[1:]):
                        nc.sync.dma_start(
                            out=xt[:, lo:hi, :], in_=xr[b][:, lo:hi, :]
                        )
                xb[b] = xt

            def stats_dve(b):
                """bn pairs + scans + PE cumsum + mean/msq/var (no sqrt)."""
                xt = xb[b]
                bnb = sb.tile([P, 2 * NPAIR, 3], F32, tag="bnb")
                for i in range(12):
                    _bn_stats_raw(
                        nc,
                        bnb[:, 2 * i:2 * i + 2, :],
                        xt[:, 2 * i:2 * i + 2, :].rearrange("p r h -> p h r"),
                    )
                # row 24 duplicated into both stream phases (stride-0 pair)
                _bn_stats_raw(
                    nc,
                    bnb[:, 24:26, :],
                    xt[:, 24:25, :]
                    .rearrange("p r h -> p h r")
                    .to_broadcast((P, H, 2)),
                )
                meansv = bnb[:, :, 1]        # [128, 26] stride-3: per-row means
                m2v = bnb[:, :, 2]           # [128, 26]: per-row M2
                ut = sb.tile([P, 2 * NPAIR], F32, tag="ut")
                nc.vector.tensor_tensor(out=ut[:], in0=meansv, in1=meansv,
                                        op=ALU.mult)
                qt = sb.tile([P, 2 * NPAIR], F32, tag="qt")
                nc.vector.scalar_tensor_tensor(
                    out=qt[:], in0=ut[:], scalar=float(H), in1=m2v,
                    op0=ALU.mult, op1=ALU.add,
                )
                St = sb.tile([P, 2 * NPAIR], F32, tag="St")
                nc.vector.tensor_tensor_scan(
                    out=St[:], data0=meansv, data1=meansv,
                    initial=0.0, op0=ALU.add, op1=ALU.bypass,
                )
                Qt = sb.tile([P, 2 * NPAIR], F32, tag="Qt")
                nc.vector.tensor_tensor_scan(
                    out=Qt[:], data0=qt[:], data1=qt[:],
                    initial=0.0, op0=ALU.add, op1=ALU.bypass,
                )
                tot = sb.tile([P, 2], F32, tag="tot")
                nc.vector.tensor_copy(out=tot[:, 0:1], in_=St[:, CL - 1:CL])
                nc.vector.tensor_copy(out=tot[:, 1:2], in_=Qt[:, CL - 1:CL])
                offs = psum.tile([P, 2], F32, tag="offs")
                nc.tensor.matmul(
                    offs[:], lhsT=sb_utri[:], rhs=tot[:], start=True, stop=False
                )
                nc.tensor.matmul(
                    offs[:], lhsT=sb_oner[:], rhs=carry[:], start=False, stop=True
                )
                btot = psum.tile([1, 2], F32, tag="btot")
                nc.tensor.matmul(
                    btot[:], lhsT=sb_onec[:], rhs=tot[:], start=True, stop=True
                )
                nc.vector.tensor_add(out=carry[:], in0=carry[:], in1=btot[:])

                # evacuate offs to SBUF: stride-0-broadcast reads straight
                # from PSUM measured 1.4-3.1 us; an SBUF scalar AP is cheap
                offs_sb = sb.tile([P, 2], F32, tag="offs_sb")
                nc.vector.tensor_copy(out=offs_sb[:], in_=offs[:])
                mean = sb.tile([P, CL], F32, tag="mean")
                nc.vector.scalar_tensor_tensor(
                    out=mean[:], in0=St[:, 0:CL], scalar=offs_sb[:, 0:1],
                    in1=sb_invm[:, b, :], op0=ALU.add, op1=ALU.mult,
                )
                msq = sb.tile([P, CL], F32, tag="msq")
                nc.vector.scalar_tensor_tensor(
                    out=msq[:], in0=Qt[:, 0:CL], scalar=offs_sb[:, 1:2],
                    in1=sb_invp[:, b, :], op0=ALU.add, op1=ALU.mult,
                )
                var = sb.tile([P, CL], F32, tag="var")
                nc.vector.tensor_tensor(out=var[:], in0=mean[:], in1=mean[:],
                                        op=ALU.mult)
                nc.vector.tensor_tensor(out=var[:], in0=msq[:], in1=var[:],
                                        op=ALU.subtract)
                varb[b] = var
                meanb[b] = mean

            def stats_act(b):
                """sqrt (ScalarE) + reciprocal + nmi (DVE)."""
                sd = sb.tile([P, CL], F32, tag="sd")
                nc.scalar.activation(
                    out=sd[:], in_=varb[b][:], func=ACTF.Sqrt, bias=sb_eps[:],
                )
                inv = sb.tile([P, CL], F32, tag="inv")
                nc.vector.reciprocal(out=inv[:], in_=sd[:])
                nmi = sb.tile([P, CL], F32, tag="nmi")
                nc.vector.scalar_tensor_tensor(
                    out=nmi[:], in0=meanb[b][:], scalar=-1.0, in1=inv[:],
                    op0=ALU.mult, op1=ALU.mult,
                )
                invb[b] = inv
                nmib[b] = nmi

            def out_band(b):
                xt = xb[b]
                inv = invb[b]
                nmi = nmib[b]
                last = b == BANDS - 1
                act_end = L_ACT_AFF_END if last else ACT_AFF_END
                dve_gam_end = L_DVE_GAM_END if last else DVE_GAM_END
                gps_gams = L_GPS_GAM_SPLITS if last else GPS_GAM_SPLITS

                for r in range(act_end):
                    nc.scalar.activation(
                        out=xt[:, r, :], in_=xt[:, r, :], func=ACTF.Identity,
                        bias=nmi[:, r:r + 1], scale=inv[:, r:r + 1],
                    )
                if last:
                    # DVE merged affine rows [13, 25) via paired-scalar bcast
                    n = CL - act_end
                    inv2 = sb.tile([P, n, 2], BF16, tag="inv2")
                    nc.vector.tensor_copy(
                        out=inv2[:],
                        in_=inv[:, act_end:CL]
                        .rearrange("p (r o) -> p r o", o=1)
                        .to_broadcast((P, n, 2)),
                    )
                    nmi2 = sb.tile([P, n, 2], BF16, tag="nmi2")
                    nc.vector.tensor_copy(
                        out=nmi2[:],
                        in_=nmi[:, act_end:CL]
                        .rearrange("p (r o) -> p r o", o=1)
                        .to_broadcast((P, n, 2)),
                    )
                    xv = xt[:, act_end:CL, :].rearrange(
                        "p r (q j) -> p r q j", j=2
                    )
                    iv = inv2[:].rearrange("p r (o j) -> p r o j", o=1)\
                        .to_broadcast((P, n, H // 2, 2))
                    nv = nmi2[:].rearrange("p r (o j) -> p r o j", o=1)\
                        .to_broadcast((P, n, H // 2, 2))
                    nc.vector.tensor_tensor(out=xv, in0=xv, in1=iv, op=ALU.mult)
                    nc.vector.tensor_tensor(out=xv, in0=xv, in1=nv, op=ALU.add)
                else:
                    for r in range(act_end, CL):
                        nc.gpsimd.tensor_scalar(
                            out=xt[:, r, :], in0=xt[:, r, :],
                            scalar1=inv[:, r:r + 1], scalar2=nmi[:, r:r + 1],
                            op0=ALU.mult, op1=ALU.add,
                        )

                # gamma
                n = dve_gam_end
                nc.vector.tensor_tensor(
                    out=xt[:, 0:n, :], in0=xt[:, 0:n, :], in1=gamma_bc(n),
                    op=ALU.mult,
                )
                if use_beta:
                    nc.vector.tensor_tensor(
                        out=xt[:, 0:n, :], in0=xt[:, 0:n, :], in1=beta_bc(n),
                        op=ALU.add,
                    )
                for lo, hi in gps_gams:
                    nc.gpsimd.tensor_tensor(
                        out=xt[:, lo:hi, :], in0=xt[:, lo:hi, :],
                        in1=gamma_bc(hi - lo), op=ALU.mult,
                    )
                    if use_beta:
                        nc.gpsimd.tensor_tensor(
                            out=xt[:, lo:hi, :], in0=xt[:, lo:hi, :],
                            in1=beta_bc(hi - lo), op=ALU.add,
                        )

            def store_band(b):
                # sync queue: triggers there don't block the ACT pipeline,
                # and each is emitted after the next band's load trigger
                xt = xb[b]
                s = STORE_SPLIT
                nc.sync.dma_start(out=yr[b][:, 0:s, :], in_=xt[:, 0:s, :])
                nc.sync.dma_start(out=yr[b][:, s:CL, :], in_=xt[:, s:CL, :])

            load_band(0, chunks=4)
            load_band(1)
            stats_dve(0)
            stats_act(0)
            for b in range(1, BANDS):
                if b + 1 < BANDS:
                    load_band(b + 1)
                stats_dve(b)
                out_band(b - 1)
                stats_act(b)
                store_band(b - 1)
            out_band(BANDS - 1)
            store_band(BANDS - 1)

    nc.compile()
    return nc


_CACHE = {}


def _get(use_beta: bool):
    if use_beta not in _CACHE:
        _CACHE[use_beta] = _build(use_beta)
    return _CACHE[use_beta]


def _make_consts():
    # strictly-upper triangular ones: lhsT[q, p] = 1 iff q < p
    utri = np.triu(np.ones((P, P), dtype=np.float32), k=1)
    ones_col = np.ones((P, 1), dtype=np.float32)
    ones_row = np.ones((1, P), dtype=np.float32)
    k = np.arange(K, dtype=np.float64).reshape(BANDS, P, CL)  # [b, p, r]
    counts = np.transpose(k, (1, 0, 2)) + 1.0                 # [p, b, r]
    invc_m = (1.0 / counts).astype(np.float32)
    invc_p = (1.0 / (float(H) * counts)).astype(np.float32)
    return utri, ones_col, ones_row, invc_m, invc_p


def _prepare(inputs, gamma, beta):
    inputs = np.asarray(inputs, dtype=np.float32)
    gamma = np.asarray(gamma, dtype=np.float32).reshape(1, H)
    beta = np.asarray(beta, dtype=np.float32).reshape(1, H)
    use_beta = bool(np.any(beta))

    xbf = inputs.astype(ml_dtypes.bfloat16)
    gamma_b = np.ascontiguousarray(
        np.broadcast_to(gamma, (P, H))
    ).astype(ml_dtypes.bfloat16)
    utri, ones_col, ones_row, invc_m, invc_p = _make_consts()

    in_maps = []
    for b in range(B):
        m = {
            "x": np.ascontiguousarray(xbf[b]),
            "gamma_b": gamma_b,
            "utri": utri,
            "ones_col": ones_col,
            "ones_row": ones_row,
            "invc_m": invc_m,
            "invc_p": invc_p,
        }
        if use_beta:
            m["beta_b"] = np.ascontiguousarray(
                np.broadcast_to(beta, (P, H))
            ).astype(ml_dtypes.bfloat16)
        in_maps.append(m)
    return use_beta, in_maps


def kernel(inputs: np.ndarray, gamma: np.ndarray, beta: np.ndarray) -> np.ndarray:
    use_beta, in_maps = _prepare(inputs, gamma, beta)
    nc = _get(use_beta)
    res = run_bass_kernel_spmd(nc, in_maps, list(range(B)))
    out = np.stack(
        [np.asarray(res.results[b]["y"]).astype(np.float32) for b in range(B)],
        axis=0,
    )
    return out
